# revision 19
# baseline (speedup 1.0000x reference)
"""BridgeNetUp KNN kernel on 8 Trainium2 NeuronCores.

Data-parallel over the batch (B=16 -> 2 samples per core), per the sharding
hint. Each core computes cdist/top-k/gather-interpolation and both pointwise
conv layers for its shard on-device (via the PJRT backend on the 8
NeuronCores). BatchNorm uses global batch statistics: each phase returns its
shard's per-channel sum/sumsq (256+256 floats), which are summed on the host
(the tiny cross-device all-reduce) and fed to the next phase; all heavy
compute and all activations stay resident on the NeuronCores between phases.

A hand-written Bass implementation of this same pipeline (fp16 hi/lo-split
score matmul + DVE max8 top-k + exact f32 re-rank + indirect-DMA gather +
PE-transpose interp + fp32r convs) is in bass_attempt.py; it validates
bit-exactly in CoreSim but two required primitives (indirect-DMA gather and
the GPSIMD custom-op library load) miscompile or misbehave on this
container's walrus/runtime, so the PJRT path below is used for the
hardware run.
"""

import time
from types import SimpleNamespace

import numpy as np

B, S, N, C1, C2, H = 16, 1024, 4096, 256, 128, 256
NCORES = 8
NB = B // NCORES
K = 3
BN_EPS = 1e-5
CNT = float(B * N)

_cache = {}


def _build_fns():
    import jax
    import jax.numpy as jnp
    from jax import lax

    def knn_concat(points1, points2, xyz1, xyz2, w1):
        d2 = jnp.sum(
            (xyz2[:, :, None, :] - xyz1[:, None, :, :]) ** 2, axis=-1)
        neg_d2k, idx = lax.top_k(-d2, K)
        d2k = -neg_d2k
        w = 1.0 / jnp.maximum(d2k, 1e-16)
        gathered = jax.vmap(lambda f, i: f[i])(points1, idx)
        interp = (jnp.sum(w[..., None] * gathered, axis=2)
                  / jnp.sum(w, axis=-1, keepdims=True))
        x = jnp.concatenate([interp, points2], axis=-1)     # [nb,N,Cin]
        y = jnp.einsum('oc,bnc->bon', w1, x)                # [nb,H,N]
        s1 = jnp.sum(y, axis=(0, 2))
        s2 = jnp.sum(y * y, axis=(0, 2))
        return y, s1, s2

    def bn_conv2(y, a1, c1, w2):
        yh = jnp.maximum(y * a1[None, :, None] + c1[None, :, None], 0.0)
        y2 = jnp.einsum('oc,bcn->bon', w2, yh)
        s1 = jnp.sum(y2, axis=(0, 2))
        s2 = jnp.sum(y2 * y2, axis=(0, 2))
        return y2, s1, s2

    def bn_out(y2, a2, c2):
        o = jnp.maximum(y2 * a2[None, :, None] + c2[None, :, None], 0.0)
        return jnp.transpose(o, (0, 2, 1))                  # [nb,N,H]

    devs = jax.devices()[:NCORES]
    p1 = jax.pmap(knn_concat, devices=devs)
    p2 = jax.pmap(bn_conv2, devices=devs)
    p3 = jax.pmap(bn_out, devices=devs)
    return p1, p2, p3


def _bn_affine(s1, s2, g, be):
    mean = s1 / CNT
    var = s2 / CNT - mean * mean
    rstd = 1.0 / np.sqrt(var + BN_EPS)
    a = (g * rstd).astype(np.float32)
    c = (be - g * rstd * mean).astype(np.float32)
    return a, c


def _run_phases(p1, p2, p3, inputs):
    import jax

    def shard(x):
        return np.ascontiguousarray(
            x.reshape(NCORES, NB, *x.shape[1:]).astype(np.float32))

    def rep(x):
        return np.ascontiguousarray(
            np.broadcast_to(x.astype(np.float32),
                            (NCORES,) + x.shape))

    y, s1, s2 = p1(shard(inputs['points1']), shard(inputs['points2']),
                   shard(inputs['xyz1']), shard(inputs['xyz2']),
                   rep(inputs['w1']))
    s1h = np.asarray(s1).sum(0)
    s2h = np.asarray(s2).sum(0)
    a1, c1 = _bn_affine(s1h, s2h, inputs['g1'], inputs['be1'])
    y2, t1, t2 = p2(y, rep(a1), rep(c1), rep(inputs['w2']))
    t1h = np.asarray(t1).sum(0)
    t2h = np.asarray(t2).sum(0)
    a2, c2 = _bn_affine(t1h, t2h, inputs['g2'], inputs['be2'])
    out = p3(y2, rep(a2), rep(c2))
    jax.block_until_ready(out)
    return np.asarray(out).reshape(B, N, H).astype(np.float32)


def run(inputs, trace=False):
    if 'fns' not in _cache:
        _cache['fns'] = _build_fns()
    p1, p2, p3 = _cache['fns']
    inputs = {k: np.asarray(v) for k, v in inputs.items()}

    t0 = time.time()
    out = _run_phases(p1, p2, p3, inputs)
    first_ns = int((time.time() - t0) * 1e9)

    # warm timed pass
    t0 = time.time()
    out = _run_phases(p1, p2, p3, inputs)
    warm_ns = int((time.time() - t0) * 1e9)

    res = SimpleNamespace(exec_time_ns=warm_ns, mean_exec_time_ns=warm_ns,
                          max_exec_time_core_id=0,
                          instructions_and_trace=None, first_ns=first_ns)
    return out, res


def kernel(**inputs):
    out, _ = run(inputs, trace=False)
    return out
